# revision 1
# baseline (speedup 1.0000x reference)
"""Cross-modal attention kernel for Trainium2 -- data-parallel over batch on 8 cores.

Reference computation per sample (C=256, H=W=64, N=H*W=4096, dqk=32):
    q = Wq @ x + bq; k = Wk @ y + bk; v = Wv @ y + bv
    out = gamma * (v @ softmax_j(q^T k)^T) + x

Strategy (per core = one batch sample):
  - Projections run in float32r, attention in bf16/fp8 so PE matmuls stream
    at 1 cycle/row (fp32 would be 4).
  - Energy is computed TRANSPOSED (E^T[j,i], keys on partitions) so the
    attention-weighted sum contracts over the partition dim with no
    transposes.  exp() is applied unnormalized (logits are O(1) by
    construction: gain-0.02 weights), softmax normalization happens on the
    [C, IBLK] output instead of the [N, N] matrix.
  - The K=32 energy matmuls are 4-way row-packed (tile_position).
  - exp(E^T) and v^T are stored fp8e4m3; AV and the denominator both run as
    MatmulPerfMode.DoubleRow contractions (2 fp8 weights/PE cell), pairing
    consecutive j-tiles via 3D [K,2,N] APs.  The denominator is a DoubleRow
    ones-matmul accumulating sum_j exp(E^T)[j,i] in PSUM.
  - Software pipelining: AV for group g-2 issues after the energy matmuls of
    group g; block n's normalization tail is deferred into block n+1.

Differences from the bf16 version:
  - exp(E^T) and v^T are stored as fp8e4m3; the AV contraction runs in
    MatmulPerfMode.DoubleRow (2 fp8 weights per PE cell -> half the cycles),
    pairing consecutive j-tiles along the partition dim via 3D [K,2,N] APs.
  - The softmax denominator is ALSO a DoubleRow matmul: ones[128,2,128] as
    stationary -> den[i] accumulates sum_j exp(E^T)[j,i] in PSUM, which
    removes the whole DVE accumulate+fold chain of the bf16 version.
  - gamma is applied as a per-partition tensor_scalar multiply on 1/den.
"""

import sys

if "/opt/trn_rl_repo" not in sys.path:
    sys.path.insert(0, "/opt/trn_rl_repo")

import numpy as np

import concourse.bacc as bacc
import concourse.mybir as mybir
import concourse.tile as tile
from concourse.bass_utils import run_bass_kernel_spmd

F32 = mybir.dt.float32
F32R = mybir.dt.float32r
BF16 = mybir.dt.bfloat16
FP8 = mybir.dt.float8e4

B, C, HW, D = 8, 256, 4096, 32
CH = C // 128
IBLK = 512
NIB = HW // IBLK
NJT = HW // 128
NPAIR = NJT // 2
EXPF = mybir.ActivationFunctionType.Exp
MULT = mybir.AluOpType.mult
ADD = mybir.AluOpType.add
DROW = mybir.MatmulPerfMode.DoubleRow


def _build():
    nc = bacc.Bacc("TRN2", target_bir_lowering=False, debug=False, num_devices=8)

    xr = nc.dram_tensor("xr", [C, HW], F32R, kind="ExternalInput")
    xf = nc.dram_tensor("xf", [C, HW], F32, kind="ExternalInput")
    yr = nc.dram_tensor("yr", [C, HW], F32R, kind="ExternalInput")
    wqT = nc.dram_tensor("wqT", [C, D], F32R, kind="ExternalInput")
    wkT = nc.dram_tensor("wkT", [C, D], F32R, kind="ExternalInput")
    wvT = nc.dram_tensor("wvT", [C, C], F32R, kind="ExternalInput")
    bqd = nc.dram_tensor("bqd", [D, 1], F32, kind="ExternalInput")
    bkd = nc.dram_tensor("bkd", [D, 1], F32, kind="ExternalInput")
    gbvd = nc.dram_tensor("gbvd", [128, CH], F32, kind="ExternalInput")
    gmd = nc.dram_tensor("gmd", [128, 1], F32, kind="ExternalInput")
    out = nc.dram_tensor("out", [C, HW], F32, kind="ExternalOutput")

    tc = tile.TileContext(nc)
    with tc:
        with (
            tc.tile_pool(name="cst", bufs=1) as cst,
            tc.tile_pool(name="qkv", bufs=1) as qkv,
        ):
            wq_sb = cst.tile([128, CH * D], F32R)
            wk_sb = cst.tile([128, CH * D], F32R)
            wv_sb = cst.tile([128, CH * C], F32R)
            bq_sb = cst.tile([D, 1], F32)
            bk_sb = cst.tile([D, 1], F32)
            gbv_sb = cst.tile([128, CH], F32)
            gm_sb = cst.tile([128, 1], F32)
            ones_sb = cst.tile([128, 2 * 128], FP8)
            nc.vector.memset(ones_sb[:], 1.0)
            nc.gpsimd.dma_start(bq_sb[:], bqd[:])
            nc.gpsimd.dma_start(bk_sb[:], bkd[:])
            nc.gpsimd.dma_start(gbv_sb[:], gbvd[:])
            nc.gpsimd.dma_start(gm_sb[:], gmd[:])

            q4 = qkv.tile([128, HW], BF16)
            k4 = qkv.tile([128, HW], BF16)
            vt = qkv.tile([128, NJT * C], FP8)

            NG = NJT // 4
            ptp = None  # assigned when the phase-B pools open
            psE = None

            def et_group(n, g, pt):
                # energy for (i-block n, group g): 4 row-packed K=32 matmuls
                # into two 2-bank psum tiles, then exp into pt (fp8)
                ets = [
                    psE.tile([128, 2 * IBLK], F32,
                             name=f"et{h}_{n}_{g}", tag="et", bufs=2)
                    for h in range(2)
                ]
                for q in range(4):
                    jt = 4 * g + q
                    nc.tensor.matmul(
                        ets[q // 2][:, (q % 2) * IBLK:(q % 2 + 1) * IBLK],
                        k4[32 * q:32 * (q + 1), jt * 128:(jt + 1) * 128],
                        q4[32 * q:32 * (q + 1), n * IBLK:(n + 1) * IBLK],
                        start=True,
                        stop=True,
                        tile_position=(32 * q, 0),
                    )
                for h in range(2):
                    nc.scalar.activation(
                        pt[:, (4 * g + 2 * h) * IBLK:(4 * g + 2 * h + 2) * IBLK],
                        ets[h][:], EXPF,
                    )

            with (
                tc.tile_pool(name="xy", bufs=1) as xy,
                tc.tile_pool(name="psA", bufs=4, space="PSUM") as psA,
            ):
                xr_sb = xy.tile([128, CH * HW], F32R)
                yr_sb = xy.tile([128, CH * HW], F32R)

                def in_chunk(src, dst_sb, h, c0, c1):
                    nc.sync.dma_start(
                        dst_sb[:, h * HW + c0: h * HW + c1],
                        src[h * 128:(h + 1) * 128, c0:c1],
                    )

                for h in range(CH):
                    nc.sync.dma_start(wq_sb[:, h * D:(h + 1) * D], wqT[h * 128:(h + 1) * 128, :])
                for h in range(CH):
                    in_chunk(xr, xr_sb, h, 0, IBLK)
                for h in range(CH):
                    nc.sync.dma_start(wk_sb[:, h * D:(h + 1) * D], wkT[h * 128:(h + 1) * 128, :])
                for h in range(CH):
                    in_chunk(yr, yr_sb, h, 0, IBLK)
                for h in range(CH):
                    nc.sync.dma_start(wv_sb[:, h * C:(h + 1) * C], wvT[h * 128:(h + 1) * 128, :])
                for ic in range(1, NIB):
                    c0, c1 = ic * IBLK, (ic + 1) * IBLK
                    for h in range(CH):
                        in_chunk(xr, xr_sb, h, c0, c1)
                        in_chunk(yr, yr_sb, h, c0, c1)
                for ic in range(NIB):
                    c0, c1 = ic * IBLK, (ic + 1) * IBLK
                    for w_sb, b_sb, src, dst in (
                        (wq_sb, bq_sb, xr_sb, q4),
                        (wk_sb, bk_sb, yr_sb, k4),
                    ):
                        ps = psA.tile([D, IBLK], F32, name=f"qk_{ic}", tag="qk_ps")
                        for h in range(CH):
                            nc.tensor.matmul(
                                ps[:],
                                w_sb[:, h * D:(h + 1) * D],
                                src[:, h * HW + c0: h * HW + c1],
                                start=(h == 0),
                                stop=(h == CH - 1),
                            )
                        nc.vector.tensor_scalar_add(
                            dst[0:D, c0:c1], ps[:], b_sb[:, 0:1]
                        )
                        for g in range(1, 4):
                            nc.gpsimd.dma_start(
                                dst[32 * g:32 * (g + 1), c0:c1], dst[0:D, c0:c1]
                            )
                    for jt in range(4 * ic, 4 * ic + 4):
                        ps = psA.tile([128, C], F32, name=f"vt_{jt}", tag="vt_ps")
                        for h in range(CH):
                            nc.tensor.matmul(
                                ps[:],
                                yr_sb[:, h * HW + jt * 128: h * HW + (jt + 1) * 128],
                                wv_sb[:, h * C:(h + 1) * C],
                                start=(h == 0),
                                stop=(h == CH - 1),
                            )
                        nc.vector.tensor_copy(vt[:, jt * C:(jt + 1) * C], ps[:])

            with (
                tc.tile_pool(name="ptp", bufs=2) as ptp,
                tc.tile_pool(name="wrk", bufs=2) as wrk,
                tc.tile_pool(name="psE", bufs=1, space="PSUM") as psE,
                tc.tile_pool(name="psAV", bufs=1, space="PSUM") as psAV,
            ):
                def make_tail(n, av, den):
                    def tail():
                        rgb = wrk.tile([128, IBLK], F32, name=f"rgb_{n}", tag="rgb")
                        nc.vector.reciprocal(rgb[:], den[:])
                        rgbg = wrk.tile([128, IBLK], F32, name=f"rgbg_{n}", tag="rgbg")
                        nc.vector.tensor_scalar(
                            rgbg[:], rgb[:], gm_sb[:, 0:1], None, MULT
                        )
                        for ch in range(CH):
                            xf_t = wrk.tile([128, IBLK], F32,
                                            name=f"xf_{n}_{ch}", tag="xf")
                            nc.sync.dma_start(
                                xf_t[:],
                                xf[ch * 128:(ch + 1) * 128, n * IBLK:(n + 1) * IBLK],
                            )
                            tmp = wrk.tile([128, IBLK], F32,
                                           name=f"tmp_{n}_{ch}", tag="tmp")
                            nc.vector.tensor_tensor(tmp[:], av[ch][:], rgbg[:], MULT)
                            ot = wrk.tile([128, IBLK], F32, name=f"ot_{n}_{ch}", tag="ot")
                            nc.vector.scalar_tensor_tensor(
                                ot[:], tmp[:], gbv_sb[:, ch:ch + 1], xf_t[:], ADD, ADD
                            )
                            nc.sync.dma_start(
                                out[ch * 128:(ch + 1) * 128, n * IBLK:(n + 1) * IBLK],
                                ot[:],
                            )
                    return tail

                ones_pair = ones_sb[:].rearrange("P (s c) -> P s c", s=2)

                pending_tail = None
                for n in range(NIB):
                    pt = ptp.tile([128, NJT * IBLK], FP8, name=f"pt_{n}", tag="pt")
                    av = [
                        psAV.tile([128, IBLK], F32, name=f"av{ch}_{n}", tag=f"av{ch}")
                        for ch in range(CH)
                    ]
                    den = psAV.tile([128, IBLK], F32, name=f"den_{n}", tag="den")

                    def av_pairs(g, pt=pt, av=av, den=den, n=n):
                        # DoubleRow AV + denominator for the 2 j-tile pairs of
                        # group g: virtual K=256 contracts two j-tiles at once
                        for p in (2 * g, 2 * g + 1):
                            ptp_ap = pt[:, 2 * p * IBLK:(2 * p + 2) * IBLK].rearrange(
                                "P (s N) -> P s N", s=2
                            )
                            vtp_ap = vt[:, 2 * p * C:(2 * p + 2) * C].rearrange(
                                "P (s c) -> P s c", s=2
                            )
                            for ch in range(CH):
                                nc.tensor.matmul(
                                    av[ch][:],
                                    vtp_ap[:, :, ch * 128:(ch + 1) * 128],
                                    ptp_ap,
                                    start=(p == 0),
                                    stop=(p == NPAIR - 1),
                                    perf_mode=DROW,
                                    skip_group_check=True,
                                )
                            nc.tensor.matmul(
                                den[:],
                                ones_pair,
                                ptp_ap,
                                start=(p == 0),
                                stop=(p == NPAIR - 1),
                                perf_mode=DROW,
                                skip_group_check=True,
                            )

                    for g in range(NG):
                        et_group(n, g, pt)
                        if g == 0 and pending_tail is not None:
                            pending_tail()
                            pending_tail = None
                        if g >= 2:
                            av_pairs(g - 2)
                    av_pairs(NG - 2)
                    av_pairs(NG - 1)
                    pending_tail = make_tail(n, av, den)
                pending_tail()
    nc.compile()
    return nc


_NC_CACHE = {}


def kernel(x, y, Wq, bq, Wk, bk, Wv, bv, gamma):
    assert x.shape == (B, C, 64, 64)
    xs = np.ascontiguousarray(x.reshape(B, C, HW).astype(np.float32))
    ys = np.ascontiguousarray(y.reshape(B, C, HW).astype(np.float32))
    wqT = np.ascontiguousarray(Wq.T.astype(np.float32))
    wkT = np.ascontiguousarray(Wk.T.astype(np.float32))
    wvT = np.ascontiguousarray(Wv.T.astype(np.float32))
    bqh = np.ascontiguousarray(bq.astype(np.float32).reshape(D, 1))
    bkh = np.ascontiguousarray(bk.astype(np.float32).reshape(D, 1))
    g = float(np.asarray(gamma).reshape(-1)[0])
    gbvh = np.ascontiguousarray((g * bv.astype(np.float32)).reshape(CH, 128).T)
    gmh = np.full((128, 1), g, dtype=np.float32)

    if "nc" not in _NC_CACHE:
        _NC_CACHE["nc"] = _build()
    nc = _NC_CACHE["nc"]

    in_maps = [
        {
            "xr": xs[b], "xf": xs[b], "yr": ys[b],
            "wqT": wqT, "wkT": wkT, "wvT": wvT,
            "bqd": bqh, "bkd": bkh, "gbvd": gbvh, "gmd": gmh,
        }
        for b in range(B)
    ]
    res = run_bass_kernel_spmd(nc, in_maps, list(range(B)))
    outs = np.stack([res.results[b]["out"] for b in range(B)])
    return outs.reshape(B, C, 64, 64).astype(np.float32)



# revision 2
# speedup vs baseline: 1.0153x; 1.0153x over previous
"""Cross-modal attention kernel for Trainium2 -- data-parallel over batch on 8 cores.

Reference computation per sample (C=256, H=W=64, N=H*W=4096, dqk=32):
    q = Wq @ x + bq; k = Wk @ y + bk; v = Wv @ y + bv
    out = gamma * (v @ softmax_j(q^T k)^T) + x

The Act engine's exp over the full 4096x4096 energy matrix is the binding
resource (~133us busy at 0.833ns/col on [128, free] tiles, dtype-independent),
so the kernel is organized as one continuous act stream with everything else
scheduled underneath it:

  - No separate projection phase: q/k/v projections are interleaved into the
    attention pipeline's PE-queue slack as deadline-ordered "filler" work.
    The first exp fires ~6us in (vs ~34us for a phased design).
  - Wq/Wk are loaded pre-replicated 4x along the output dim, so one matmul
    per channel-chunk directly yields q/k in the 4-row-group layout that the
    4-way tile_position energy packing wants (no broadcast copies at all).
  - Energy is computed TRANSPOSED (E^T[j,i], keys on partitions); exp is
    applied unnormalized to fp8e4m3; AV and the softmax denominator are
    MatmulPerfMode.DoubleRow fp8 contractions (2 j-tiles per pass); the
    denominator's normalization happens on the [C, IBLK] output.
  - The exp act table is preloaded at t=0 by a dummy 1-element activation.
  - The residual x is re-read from the f32r SBUF projection operand via
    bitcast -- x is DMA'd once, not twice.
  - Tail (1/den, *gamma, +x, store) is split DVE/Pool so the two channel
    chunks normalize in parallel; within each AV pair the den matmul issues
    first so the final block's tail starts as early as possible.

PSUM budget (8 banks of 2KB): energy ring 2x[128,1024]f32 = 4, av accums
2x[128,512] = 2, den 1, projection scratch [128,512] = 1.
"""

import sys

if "/opt/trn_rl_repo" not in sys.path:
    sys.path.insert(0, "/opt/trn_rl_repo")

import numpy as np

import concourse.bacc as bacc
import concourse.mybir as mybir
import concourse.tile as tile
from concourse.bass_utils import run_bass_kernel_spmd

F32 = mybir.dt.float32
F32R = mybir.dt.float32r
BF16 = mybir.dt.bfloat16
FP8 = mybir.dt.float8e4

B, C, HW, D = 8, 256, 4096, 32
CH = C // 128
IBLK = 512
NIB = HW // IBLK
NJT = HW // 128
NPAIR = NJT // 2
NG = NJT // 4
EXPF = mybir.ActivationFunctionType.Exp
MULT = mybir.AluOpType.mult
ADD = mybir.AluOpType.add
DROW = mybir.MatmulPerfMode.DoubleRow


def _build():
    nc = bacc.Bacc("TRN2", target_bir_lowering=False, debug=False, num_devices=8)

    xr = nc.dram_tensor("xr", [C, HW], F32R, kind="ExternalInput")
    yr = nc.dram_tensor("yr", [C, HW], F32R, kind="ExternalInput")
    wq4d = nc.dram_tensor("wq4", [C, 128], F32R, kind="ExternalInput")
    wk4d = nc.dram_tensor("wk4", [C, 128], F32R, kind="ExternalInput")
    wvd = nc.dram_tensor("wvT", [C, C], F32R, kind="ExternalInput")
    bq4d = nc.dram_tensor("bq4", [128, 1], F32, kind="ExternalInput")
    bk4d = nc.dram_tensor("bk4", [128, 1], F32, kind="ExternalInput")
    gbvd = nc.dram_tensor("gbvd", [128, CH], F32, kind="ExternalInput")
    gmd = nc.dram_tensor("gmd", [128, 1], F32, kind="ExternalInput")
    out = nc.dram_tensor("out", [C, HW], F32, kind="ExternalOutput")

    tc = tile.TileContext(nc)
    with tc:
        with (
            tc.tile_pool(name="cst", bufs=1) as cst,
            tc.tile_pool(name="io", bufs=1) as io,
            tc.tile_pool(name="qkv", bufs=1) as qkv,
        ):
            wq4_sb = cst.tile([128, CH * 128], F32R)
            wk4_sb = cst.tile([128, CH * 128], F32R)
            wv_sb = cst.tile([128, CH * C], F32R)
            bq4_sb = cst.tile([128, 1], F32)
            bk4_sb = cst.tile([128, 1], F32)
            gbv_sb = cst.tile([128, CH], F32)
            gm_sb = cst.tile([128, 1], F32)
            ones_sb = cst.tile([128, 2 * 128], FP8)
            scr = cst.tile([1, 2], F32)

            xr_sb = io.tile([128, CH * HW], F32R)
            yr_sb = io.tile([128, CH * HW], F32R)
            q4 = qkv.tile([128, HW], BF16)
            k4 = qkv.tile([128, HW], BF16)
            vt = qkv.tile([128, NJT * C], FP8)

            # exp act-table preload: first act-engine instruction, no deps
            # beyond the memset, so LoadActFuncSet runs at t~0.
            nc.vector.memset(ones_sb[:], 1.0)
            nc.scalar.activation(scr[:, 0:1], ones_sb[0:1, 0:1], EXPF)

            # weights via the gpsimd SWDGE queue, in parallel with the
            # HWDGE input stream below
            nc.gpsimd.dma_start(wk4_sb[:, 0:128], wk4d[0:128, :])
            nc.gpsimd.dma_start(wk4_sb[:, 128:256], wk4d[128:256, :])
            nc.gpsimd.dma_start(wq4_sb[:, 0:128], wq4d[0:128, :])
            nc.gpsimd.dma_start(wq4_sb[:, 128:256], wq4d[128:256, :])
            nc.gpsimd.dma_start(bk4_sb[:], bk4d[:])
            nc.gpsimd.dma_start(bq4_sb[:], bq4d[:])
            nc.gpsimd.dma_start(gbv_sb[:], gbvd[:])
            nc.gpsimd.dma_start(gm_sb[:], gmd[:])

            # inputs on the sync/HWDGE queue, j-deadline order: y chunk c
            # feeds k-chunk c (needed at block-0 slot c) and vt chunk c.
            yr3 = yr.rearrange("(ch p) N -> p ch N", ch=CH)
            xr3 = xr.rearrange("(ch p) N -> p ch N", ch=CH)
            yr_sb3 = yr_sb[:].rearrange("P (ch N) -> P ch N", ch=CH)
            xr_sb3 = xr_sb[:].rearrange("P (ch N) -> P ch N", ch=CH)

            def ld(dst3, src3, c0, c1):
                nc.sync.dma_start(dst3[:, :, c0:c1], src3[:, :, c0:c1])

            ld(yr_sb3, yr3, 0, IBLK)
            ld(xr_sb3, xr3, 0, IBLK)
            nc.sync.dma_start(wv_sb[:, 0:C], wvd[0:128, :])
            nc.sync.dma_start(wv_sb[:, C:2 * C], wvd[128:256, :])
            for c in range(1, NIB):
                ld(yr_sb3, yr3, c * IBLK, (c + 1) * IBLK)
            for n in range(1, NIB):
                ld(xr_sb3, xr3, n * IBLK, (n + 1) * IBLK)

            with (
                tc.tile_pool(name="ptp", bufs=2) as ptp,
                tc.tile_pool(name="wrk", bufs=2) as wrk,
                tc.tile_pool(name="psE", bufs=1, space="PSUM") as psE,
                tc.tile_pool(name="psAV", bufs=1, space="PSUM") as psAV,
                tc.tile_pool(name="psPR", bufs=1, space="PSUM") as psPR,
            ):
                psa = psPR.tile([128, IBLK], F32, name="psa")
                vhalf = [0]

                def k_chunk(c, w_sb=wk4_sb, b_sb=bk4_sb, src=yr_sb, dst=k4):
                    for h in range(CH):
                        nc.tensor.matmul(
                            psa[:],
                            w_sb[:, h * 128:(h + 1) * 128],
                            src[:, h * HW + c * IBLK: h * HW + (c + 1) * IBLK],
                            start=(h == 0),
                            stop=(h == CH - 1),
                        )
                    nc.vector.tensor_scalar_add(
                        dst[:, c * IBLK:(c + 1) * IBLK], psa[:], b_sb[:, 0:1]
                    )

                def q_block(n):
                    k_chunk(n, wq4_sb, bq4_sb, xr_sb, q4)

                def vt_tile(jt):
                    # ping-pong the two halves of the scratch bank so the
                    # copy of tile i overlaps the matmuls of tile i+1
                    h0 = (vhalf[0] & 1) * 256
                    vhalf[0] += 1
                    ps = psa[:, h0:h0 + 256]
                    for h in range(CH):
                        nc.tensor.matmul(
                            ps,
                            yr_sb[:, h * HW + jt * 128: h * HW + (jt + 1) * 128],
                            wv_sb[:, h * C:(h + 1) * C],
                            start=(h == 0),
                            stop=(h == CH - 1),
                        )
                    nc.vector.tensor_copy(vt[:, jt * C:(jt + 1) * C], ps)

                # deadline-ordered projection filler: fill[(n, g)] issues
                # right after slot (n, g)'s energy+act, before its av_pairs.
                fill = {}
                fill[(0, 0)] = [lambda: k_chunk(1), lambda: vt_tile(0),
                                lambda: vt_tile(1)]
                fill[(0, 1)] = [lambda: k_chunk(2), lambda: vt_tile(2),
                                lambda: vt_tile(3)]
                for g in range(2, 7):
                    fill[(0, g)] = [lambda g=g: k_chunk(g + 1)] + [
                        lambda j=j: vt_tile(j)
                        for j in range(4 * g - 4, 4 * g)
                    ]
                fill[(0, 7)] = [lambda: q_block(1)] + [
                    lambda j=j: vt_tile(j) for j in range(24, 32)
                ]
                for n in range(1, 7):
                    fill[(n, 1)] = [lambda n=n: q_block(n + 1)]

                def et_group(n, g, pt):
                    # energy for (i-block n, group g): 4 row-packed K=32
                    # matmuls into two 2-bank psum tiles, then exp into pt
                    ets = [
                        psE.tile([128, 2 * IBLK], F32,
                                 name=f"et{h}_{n}_{g}", tag="et", bufs=2)
                        for h in range(2)
                    ]
                    for q in range(4):
                        jt = 4 * g + q
                        nc.tensor.matmul(
                            ets[q // 2][:, (q % 2) * IBLK:(q % 2 + 1) * IBLK],
                            k4[32 * q:32 * (q + 1), jt * 128:(jt + 1) * 128],
                            q4[32 * q:32 * (q + 1), n * IBLK:(n + 1) * IBLK],
                            start=True,
                            stop=True,
                            tile_position=(32 * q, 0),
                        )
                    for h in range(2):
                        nc.scalar.activation(
                            pt[:, (4 * g + 2 * h) * IBLK:(4 * g + 2 * h + 2) * IBLK],
                            ets[h][:], EXPF,
                        )

                ones_pair = ones_sb[:].rearrange("P (s c) -> P s c", s=2)
                out3 = out.rearrange("(ch p) N -> p ch N", ch=CH)

                def make_tail(n, av, den):
                    def tail():
                        rgb = wrk.tile([128, IBLK], F32, name=f"rgb_{n}", tag="rgb")
                        nc.vector.reciprocal(rgb[:], den[:])
                        rgbg = wrk.tile([128, IBLK], F32, name=f"rgbg_{n}", tag="rgbg")
                        nc.vector.tensor_scalar(
                            rgbg[:], rgb[:], gm_sb[:, 0:1], None, MULT
                        )
                        ot = wrk.tile([128, CH * IBLK], F32, name=f"ot_{n}", tag="ot")
                        for ch, eng in ((0, nc.vector), (1, nc.gpsimd)):
                            xres = xr_sb[
                                :, ch * HW + n * IBLK: ch * HW + (n + 1) * IBLK
                            ].bitcast(F32)
                            tmp = wrk.tile([128, IBLK], F32,
                                           name=f"tmp_{n}_{ch}", tag=f"tmp{ch}")
                            eng.tensor_tensor(tmp[:], av[ch][:], rgbg[:], MULT)
                            eng.scalar_tensor_tensor(
                                ot[:, ch * IBLK:(ch + 1) * IBLK],
                                tmp[:], gbv_sb[:, ch:ch + 1], xres, ADD, ADD,
                            )
                        nc.sync.dma_start(
                            out3[:, :, n * IBLK:(n + 1) * IBLK],
                            ot[:].rearrange("P (ch N) -> P ch N", ch=CH),
                        )
                    return tail

                pending_tail = None
                for n in range(NIB):
                    pt = ptp.tile([128, NJT * IBLK], FP8, name=f"pt_{n}", tag="pt")
                    av = [
                        psAV.tile([128, IBLK], F32, name=f"av{ch}_{n}", tag=f"av{ch}")
                        for ch in range(CH)
                    ]
                    den = psAV.tile([128, IBLK], F32, name=f"den_{n}", tag="den")

                    def av_pairs(g, pt=pt, av=av, den=den):
                        # DoubleRow AV + denominator for the 2 j-tile pairs
                        # of group g: virtual K=256 contracts two j-tiles at
                        # once.  den first so the last block's tail can start
                        # before its final av matmuls retire.
                        for p in (2 * g, 2 * g + 1):
                            ptp_ap = pt[:, 2 * p * IBLK:(2 * p + 2) * IBLK].rearrange(
                                "P (s N) -> P s N", s=2
                            )
                            vtp_ap = vt[:, 2 * p * C:(2 * p + 2) * C].rearrange(
                                "P (s c) -> P s c", s=2
                            )
                            nc.tensor.matmul(
                                den[:],
                                ones_pair,
                                ptp_ap,
                                start=(p == 0),
                                stop=(p == NPAIR - 1),
                                perf_mode=DROW,
                                skip_group_check=True,
                            )
                            for ch in range(CH):
                                nc.tensor.matmul(
                                    av[ch][:],
                                    vtp_ap[:, :, ch * 128:(ch + 1) * 128],
                                    ptp_ap,
                                    start=(p == 0),
                                    stop=(p == NPAIR - 1),
                                    perf_mode=DROW,
                                    skip_group_check=True,
                                )

                    if n == 0:
                        k_chunk(0)
                        q_block(0)
                    for g in range(NG):
                        et_group(n, g, pt)
                        if g == 0 and pending_tail is not None:
                            pending_tail()
                            pending_tail = None
                        for thunk in fill.get((n, g), ()):
                            thunk()
                        if g >= 2:
                            av_pairs(g - 2)
                    av_pairs(NG - 2)
                    av_pairs(NG - 1)
                    pending_tail = make_tail(n, av, den)
                pending_tail()
    nc.compile()
    return nc


_NC_CACHE = {}


def kernel(x, y, Wq, bq, Wk, bk, Wv, bv, gamma):
    assert x.shape == (B, C, 64, 64)
    xs = np.ascontiguousarray(x.reshape(B, C, HW).astype(np.float32))
    ys = np.ascontiguousarray(y.reshape(B, C, HW).astype(np.float32))
    wq4 = np.ascontiguousarray(np.tile(Wq.T.astype(np.float32), (1, 4)))
    wk4 = np.ascontiguousarray(np.tile(Wk.T.astype(np.float32), (1, 4)))
    wvT = np.ascontiguousarray(Wv.T.astype(np.float32))
    bq4 = np.ascontiguousarray(np.tile(bq.astype(np.float32), 4).reshape(128, 1))
    bk4 = np.ascontiguousarray(np.tile(bk.astype(np.float32), 4).reshape(128, 1))
    g = float(np.asarray(gamma).reshape(-1)[0])
    gbvh = np.ascontiguousarray((g * bv.astype(np.float32)).reshape(CH, 128).T)
    gmh = np.full((128, 1), g, dtype=np.float32)

    if "nc" not in _NC_CACHE:
        _NC_CACHE["nc"] = _build()
    nc = _NC_CACHE["nc"]

    in_maps = [
        {
            "xr": xs[b], "yr": ys[b],
            "wq4": wq4, "wk4": wk4, "wvT": wvT,
            "bq4": bq4, "bk4": bk4, "gbvd": gbvh, "gmd": gmh,
        }
        for b in range(B)
    ]
    res = run_bass_kernel_spmd(nc, in_maps, list(range(B)))
    outs = np.stack([res.results[b]["out"] for b in range(B)])
    return outs.reshape(B, C, 64, 64).astype(np.float32)


# revision 18
# speedup vs baseline: 1.0806x; 1.0643x over previous
"""Cross-modal attention kernel for Trainium2 -- data-parallel over batch on 8 cores.

Reference computation per sample (C=256, H=W=64, N=H*W=4096, dqk=32):
    q = Wq @ x + bq; k = Wk @ y + bk; v = Wv @ y + bv
    out = gamma * (v @ softmax_j(q^T k)^T) + x

The Act engine's exp over the full 4096x4096 energy matrix is the binding
resource (~133us busy at 0.833ns/col on [128, free] tiles, dtype-independent),
so the kernel is organized as one continuous act stream with everything else
scheduled underneath it:

  - No separate projection phase: q/k/v projections are interleaved into the
    attention pipeline's PE-queue slack as deadline-ordered "filler" work.
  - Wq/Wk are loaded pre-replicated 4x along the output dim, so one matmul
    per channel-chunk directly yields q/k in the 4-row-group layout that the
    4-way tile_position energy packing wants (no broadcast copies at all).
  - Energy is computed TRANSPOSED (E^T[j,i], keys on partitions); exp is
    applied unnormalized to fp8e4m3; AV and the softmax denominator are
    MatmulPerfMode.DoubleRow fp8 contractions (2 j-tiles per pass); the
    denominator's normalization happens on the [C, IBLK] output.
  - The exp act table is preloaded at t=0 by a dummy 1-element activation;
    a burst of warmup matmuls keeps PE continuously busy from t~0.4us so it
    reaches full pstate before the first projection.
  - AV runs at lag 3 behind the energy groups and the last three AV groups
    of each block spill into the next block's first three slots, so the PE
    queue always has ready work and its idle gaps stay below the pstate
    reset threshold.
  - The residual x is re-read from the f32r SBUF projection operand via
    bitcast -- x is DMA'd once, not twice.
  - Tail (1/den, *gamma, +x, store) is split DVE/Pool so the two channel
    chunks normalize in parallel; within each AV pair the den matmul issues
    first so the final block's tail starts as early as possible.

PSUM budget (8 banks of 2KB): energy ring 2x[128,1024]f32 = 4, av accums
2x[128,512] = 2, den 1, projection scratch [128,512] = 1.
"""

import sys

if "/opt/trn_rl_repo" not in sys.path:
    sys.path.insert(0, "/opt/trn_rl_repo")

import numpy as np

import concourse.bacc as bacc
import concourse.mybir as mybir
import concourse.tile as tile
from concourse.bass_utils import run_bass_kernel_spmd

F32 = mybir.dt.float32
F32R = mybir.dt.float32r
BF16 = mybir.dt.bfloat16
FP8 = mybir.dt.float8e4

B, C, HW, D = 8, 256, 4096, 32
CH = C // 128
IBLK = 512
NIB = HW // IBLK
NJT = HW // 128
NPAIR = NJT // 2
NG = NJT // 4
NWARM = 10
EXPF = mybir.ActivationFunctionType.Exp
MULT = mybir.AluOpType.mult
ADD = mybir.AluOpType.add
DROW = mybir.MatmulPerfMode.DoubleRow


def _build():
    nc = bacc.Bacc("TRN2", target_bir_lowering=False, debug=False, num_devices=8)

    xr = nc.dram_tensor("xr", [C, HW], F32R, kind="ExternalInput")
    yr = nc.dram_tensor("yr", [C, HW], F32R, kind="ExternalInput")
    wq4d = nc.dram_tensor("wq4", [C, 128], F32R, kind="ExternalInput")
    wk4d = nc.dram_tensor("wk4", [C, 128], F32R, kind="ExternalInput")
    wvd = nc.dram_tensor("wvT", [C, C], F32R, kind="ExternalInput")
    bq4d = nc.dram_tensor("bq4", [128, 1], F32, kind="ExternalInput")
    bk4d = nc.dram_tensor("bk4", [128, 1], F32, kind="ExternalInput")
    gbvd = nc.dram_tensor("gbvd", [128, CH], F32, kind="ExternalInput")
    gmd = nc.dram_tensor("gmd", [128, 1], F32, kind="ExternalInput")
    g1md = nc.dram_tensor("g1md", [128, 1], F32, kind="ExternalInput")
    out = nc.dram_tensor("out", [C, HW], F32, kind="ExternalOutput")

    tc = tile.TileContext(nc)
    with tc:
        with (
            tc.tile_pool(name="cst", bufs=1) as cst,
            tc.tile_pool(name="io", bufs=1) as io,
            tc.tile_pool(name="qkv", bufs=1) as qkv,
        ):
            wq4_sb = cst.tile([128, CH * 128], F32R)
            wk4_sb = cst.tile([128, CH * 128], F32R)
            wv_sb = cst.tile([128, CH * C], F32R)
            bq4_sb = cst.tile([128, 1], F32)
            bk4_sb = cst.tile([128, 1], F32)
            gbv_sb = cst.tile([128, CH], F32)
            gm_sb = cst.tile([128, 1], F32)
            g1m_sb = cst.tile([128, 1], F32)
            ones_sb = cst.tile([128, 2 * 128], FP8)
            scr = cst.tile([1, 2], F32)

            xr_sb = io.tile([128, CH * HW], F32R)
            yr_sb = io.tile([128, CH * HW], F32R)
            q4 = qkv.tile([128, HW], BF16)
            k4 = qkv.tile([128, HW], BF16)
            vt = qkv.tile([128, NJT * C], FP8)

            # exp act-table preload: first act-engine instruction, no deps
            # beyond the memset, so LoadActFuncSet runs at t~0.  memset on
            # Pool (95ns) so the PE warmup below can start immediately.
            nc.gpsimd.memset(ones_sb[:], 1.0)
            nc.scalar.activation(scr[:, 0:1], ones_sb[0:1, 0:1], EXPF)

            # small constants via the gpsimd SWDGE queue; their transfers are
            # tiny and must not queue behind the big input transfers
            nc.gpsimd.dma_start(bk4_sb[:], bk4d[:])
            nc.gpsimd.dma_start(bq4_sb[:], bq4d[:])
            nc.gpsimd.dma_start(gbv_sb[:], gbvd[:])
            nc.gpsimd.dma_start(gm_sb[:], gmd[:])
            nc.gpsimd.dma_start(g1m_sb[:], g1md[:])

            # inputs on the sync/HWDGE queue in dependency-deadline order:
            # the first energy group needs wk4+y0 then wq4+x0; y chunk c
            # feeds k-chunk c (needed at block-0 slot c) and vt chunk c.
            yr3 = yr.rearrange("(ch p) N -> p ch N", ch=CH)
            xr3 = xr.rearrange("(ch p) N -> p ch N", ch=CH)
            yr_sb3 = yr_sb[:].rearrange("P (ch N) -> P ch N", ch=CH)
            xr_sb3 = xr_sb[:].rearrange("P (ch N) -> P ch N", ch=CH)

            def ld(dst3, src3, c0, c1):
                nc.sync.dma_start(dst3[:, :, c0:c1], src3[:, :, c0:c1])

            nc.sync.dma_start(wk4_sb[:, 0:128], wk4d[0:128, :])
            nc.sync.dma_start(wk4_sb[:, 128:256], wk4d[128:256, :])
            ld(yr_sb3, yr3, 0, IBLK)
            nc.sync.dma_start(wq4_sb[:, 0:128], wq4d[0:128, :])
            nc.sync.dma_start(wq4_sb[:, 128:256], wq4d[128:256, :])
            ld(xr_sb3, xr3, 0, IBLK)
            nc.sync.dma_start(wv_sb[:, 0:C], wvd[0:128, :])
            nc.sync.dma_start(wv_sb[:, C:2 * C], wvd[128:256, :])
            for c in range(1, NIB):
                ld(yr_sb3, yr3, c * IBLK, (c + 1) * IBLK)
            for n in range(1, NIB):
                ld(xr_sb3, xr3, n * IBLK, (n + 1) * IBLK)

            with (
                tc.tile_pool(name="ptp", bufs=2) as ptp,
                tc.tile_pool(name="wrk", bufs=2) as wrk,
                tc.tile_pool(name="psE", bufs=1, space="PSUM") as psE,
                tc.tile_pool(name="psAV", bufs=1, space="PSUM") as psAV,
                tc.tile_pool(name="psPR", bufs=1, space="PSUM") as psPR,
            ):
                psa = psPR.tile([128, IBLK], F32, name="psa")
                vhalf = [0]

                def k_chunk(c, w_sb=wk4_sb, b_sb=bk4_sb, src=yr_sb, dst=k4):
                    for h in range(CH):
                        nc.tensor.matmul(
                            psa[:],
                            w_sb[:, h * 128:(h + 1) * 128],
                            src[:, h * HW + c * IBLK: h * HW + (c + 1) * IBLK],
                            start=(h == 0),
                            stop=(h == CH - 1),
                        )
                    nc.vector.tensor_scalar_add(
                        dst[:, c * IBLK:(c + 1) * IBLK], psa[:], b_sb[:, 0:1]
                    )

                def q_block(n):
                    k_chunk(n, wq4_sb, bq4_sb, xr_sb, q4)

                def vt_tile(jt):
                    # ping-pong the two halves of the scratch bank so the
                    # copy of tile i overlaps the matmuls of tile i+1; the
                    # copy runs on Pool so the two halves drain through a
                    # different engine than the k/q bias-adds (DVE)
                    h0 = (vhalf[0] & 1) * 256
                    vhalf[0] += 1
                    ps = psa[:, h0:h0 + 256]
                    for h in range(CH):
                        nc.tensor.matmul(
                            ps,
                            yr_sb[:, h * HW + jt * 128: h * HW + (jt + 1) * 128],
                            wv_sb[:, h * C:(h + 1) * C],
                            start=(h == 0),
                            stop=(h == CH - 1),
                        )
                    nc.gpsimd.tensor_copy(vt[:, jt * C:(jt + 1) * C], ps)

                # deadline-ordered projection filler, split into vt tiles
                # (interleaved between other PE work so the psum scratch
                # ping-pong latency hides) and k/q chunks.
                fill_vt = {}
                fill_kq = {}
                fill_vt[(0, 0)] = [0, 1]
                fill_vt[(0, 1)] = [2, 3]
                for g in range(2, 8):
                    fill_vt[(0, g)] = list(range(3 * g - 2, 3 * g + 1))
                fill_vt[(1, 0)] = [22, 23, 24]
                fill_vt[(1, 1)] = [25, 26, 27]
                fill_vt[(1, 2)] = [28, 29, 30]
                fill_vt[(1, 3)] = [31]
                for g in range(1, 8):
                    fill_kq[(0, g - 1)] = lambda g=g: k_chunk(g)
                fill_kq[(0, 7)] = lambda: q_block(1)
                fill_kq[(1, 4)] = lambda: q_block(2)
                for n in range(2, 7):
                    fill_kq[(n, 1)] = lambda n=n: q_block(n + 1)

                def et_group(n, g, pt):
                    # energy for (i-block n, group g): 4 row-packed K=32
                    # matmuls into two 2-bank psum tiles, then exp into pt
                    ets = [
                        psE.tile([128, 2 * IBLK], F32,
                                 name=f"et{h}_{n}_{g}", tag="et", bufs=2)
                        for h in range(2)
                    ]
                    for q in range(4):
                        jt = 4 * g + q
                        nc.tensor.matmul(
                            ets[q // 2][:, (q % 2) * IBLK:(q % 2 + 1) * IBLK],
                            k4[32 * q:32 * (q + 1), jt * 128:(jt + 1) * 128],
                            q4[32 * q:32 * (q + 1), n * IBLK:(n + 1) * IBLK],
                            start=True,
                            stop=True,
                            tile_position=(32 * q, 0),
                        )
                    for h in range(2):
                        nc.scalar.activation(
                            pt[:, (4 * g + 2 * h) * IBLK:(4 * g + 2 * h + 2) * IBLK],
                            ets[h][:], EXPF,
                        )

                ones_pair = ones_sb[:].rearrange("P (s c) -> P s c", s=2)

                def make_tail(n, av, den, last=False):
                    def tail():
                        rgbg = wrk.tile([128, IBLK], F32, name=f"rgbg_{n}", tag="rgbg")
                        rgb = wrk.tile([128, IBLK], F32,
                                       name=f"rgb_{n}", tag="rgb")
                        nc.vector.reciprocal(rgb[:], den[:])
                        nc.vector.tensor_scalar(
                            rgbg[:], rgb[:], gm_sb[:, 0:1], None, MULT
                        )
                        ot = wrk.tile([128, CH * IBLK], F32, name=f"ot_{n}", tag="ot")
                        for ch, eng in ((0, nc.vector), (1, nc.gpsimd)):
                            xres = xr_sb[
                                :, ch * HW + n * IBLK: ch * HW + (n + 1) * IBLK
                            ].bitcast(F32)
                            tmp = wrk.tile([128, IBLK], F32,
                                           name=f"tmp_{n}_{ch}", tag=f"tmp{ch}")
                            eng.tensor_tensor(tmp[:], av[ch][:], rgbg[:], MULT)
                            eng.scalar_tensor_tensor(
                                ot[:, ch * IBLK:(ch + 1) * IBLK],
                                tmp[:], gbv_sb[:, ch:ch + 1], xres, ADD, ADD,
                            )
                            # per-channel store so ch0 ships while ch1 runs
                            nc.sync.dma_start(
                                out[ch * 128:(ch + 1) * 128,
                                    n * IBLK:(n + 1) * IBLK],
                                ot[:, ch * IBLK:(ch + 1) * IBLK],
                            )
                    return tail

                prev_av = None
                prev_tail = None
                for n in range(NIB):
                    pt = ptp.tile([128, NJT * IBLK], FP8, name=f"pt_{n}", tag="pt")
                    av = [
                        psAV.tile([128, IBLK], F32, name=f"av{ch}_{n}", tag=f"av{ch}")
                        for ch in range(CH)
                    ]
                    den = psAV.tile([128, IBLK], F32, name=f"den_{n}", tag="den")

                    def av_pairs(g, pt=pt, av=av, den=den):
                        # DoubleRow AV + denominator for the 2 j-tile pairs
                        # of group g: virtual K=256 contracts two j-tiles at
                        # once.  den first so the last block's tail can start
                        # before its final av matmuls retire.
                        for p in (2 * g, 2 * g + 1):
                            ptp_ap = pt[:, 2 * p * IBLK:(2 * p + 2) * IBLK].rearrange(
                                "P (s N) -> P s N", s=2
                            )
                            vtp_ap = vt[:, 2 * p * C:(2 * p + 2) * C].rearrange(
                                "P (s c) -> P s c", s=2
                            )
                            nc.tensor.matmul(
                                den[:],
                                ones_pair,
                                ptp_ap,
                                start=(p == 0),
                                stop=(p == NPAIR - 1),
                                perf_mode=DROW,
                                skip_group_check=True,
                            )
                            for ch in range(CH):
                                nc.tensor.matmul(
                                    av[ch][:],
                                    vtp_ap[:, :, ch * 128:(ch + 1) * 128],
                                    ptp_ap,
                                    start=(p == 0),
                                    stop=(p == NPAIR - 1),
                                    perf_mode=DROW,
                                    skip_group_check=True,
                                )

                    def warm(k):
                        # PE pstate warmup burst into the energy psum ring:
                        # keeps the array streaming so the projections and
                        # first energy groups run at full clock
                        wt = psE.tile([128, 2 * IBLK], F32,
                                      name=f"warm_{k}", tag="et", bufs=2)
                        nc.tensor.matmul(
                            wt[:, 0:256], ones_sb[:, 0:128], ones_sb[:],
                            start=True, stop=True,
                        )

                    if n == 0:
                        for w in range(NWARM):
                            warm(w)
                        k_chunk(0)
                        for w in range(3):
                            warm(NWARM + w)
                        q_block(0)
                    for g in range(NG):
                        et_group(n, g, pt)
                        others = []
                        if n > 0 and g <= 3:
                            # spilled av groups 4..7 of the previous block
                            others.append(
                                lambda f=prev_av, gg=NG - 4 + g: f(gg))
                        if (n, g) in fill_kq:
                            others.append(fill_kq[(n, g)])
                        if g >= 4:
                            others.append(lambda gg=g - 4: av_pairs(gg))
                        vts = [lambda j=j: vt_tile(j)
                               for j in fill_vt.get((n, g), ())]
                        # interleave vt tiles between the other work items
                        # so their psum ping-pong never head-of-line blocks
                        seq = []
                        i = 0
                        for o in others:
                            if i < len(vts):
                                seq.append(vts[i])
                                i += 1
                            seq.append(o)
                        seq.extend(vts[i:])
                        for thunk in seq:
                            thunk()
                        if n > 0 and g == 3:
                            prev_tail()
                    prev_av = av_pairs
                    prev_tail = make_tail(n, av, den, last=(n == NIB - 1))
                prev_av(NG - 4)
                prev_av(NG - 3)
                prev_av(NG - 2)
                prev_av(NG - 1)
                prev_tail()
    nc.compile()
    return nc


_NC_CACHE = {}


def kernel(x, y, Wq, bq, Wk, bk, Wv, bv, gamma):
    assert x.shape == (B, C, 64, 64)
    xs = np.ascontiguousarray(x.reshape(B, C, HW).astype(np.float32))
    ys = np.ascontiguousarray(y.reshape(B, C, HW).astype(np.float32))
    wq4 = np.ascontiguousarray(np.tile(Wq.T.astype(np.float32), (1, 4)))
    wk4 = np.ascontiguousarray(np.tile(Wk.T.astype(np.float32), (1, 4)))
    wvT = np.ascontiguousarray(Wv.T.astype(np.float32))
    bq4 = np.ascontiguousarray(np.tile(bq.astype(np.float32), 4).reshape(128, 1))
    bk4 = np.ascontiguousarray(np.tile(bk.astype(np.float32), 4).reshape(128, 1))
    g = float(np.asarray(gamma).reshape(-1)[0])
    gbvh = np.ascontiguousarray((g * bv.astype(np.float32)).reshape(CH, 128).T)
    gmh = np.full((128, 1), g, dtype=np.float32)
    g1mh = np.full((128, 1), 1.0 / g if g != 0.0 else 0.0, dtype=np.float32)

    if "nc" not in _NC_CACHE:
        _NC_CACHE["nc"] = _build()
    nc = _NC_CACHE["nc"]

    in_maps = [
        {
            "xr": xs[b], "yr": ys[b],
            "wq4": wq4, "wk4": wk4, "wvT": wvT,
            "bq4": bq4, "bk4": bk4, "gbvd": gbvh, "gmd": gmh, "g1md": g1mh,
        }
        for b in range(B)
    ]
    res = run_bass_kernel_spmd(nc, in_maps, list(range(B)))
    outs = np.stack([res.results[b]["out"] for b in range(B)])
    return outs.reshape(B, C, 64, 64).astype(np.float32)


# revision 32
# speedup vs baseline: 1.1765x; 1.0887x over previous
"""Cross-modal attention kernel for Trainium2 -- data-parallel over batch on 8 cores.

Reference computation per sample (C=256, H=W=64, N=H*W=4096, dqk=32):
    q = Wq @ x + bq; k = Wk @ y + bk; v = Wv @ y + bv
    out = gamma * (v @ softmax_j(q^T k)^T) + x

The Act engine's exp over the full 4096x4096 energy matrix is the binding
resource (~133us busy at 0.833ns/col on [128, free] tiles, dtype-independent),
so the kernel is organized as one continuous act stream with everything else
scheduled underneath it:

  - No separate projection phase: q/k/v projections are interleaved into the
    attention pipeline's PE-queue slack as deadline-ordered "filler" work.
  - Wq/Wk are loaded pre-replicated 4x along the output dim, so one matmul
    per channel-chunk directly yields q/k in the 4-row-group layout that the
    4-way tile_position energy packing wants (no broadcast copies at all).
  - Energy is computed TRANSPOSED (E^T[j,i], keys on partitions); exp is
    applied unnormalized to fp8e4m3; AV and the softmax denominator are
    MatmulPerfMode.DoubleRow fp8 contractions (2 j-tiles per pass); the
    denominator's normalization happens on the [C, IBLK] output.
  - The exp act table is preloaded at t=0 by a dummy 1-element activation;
    a burst of warmup matmuls keeps PE continuously busy from t~0.4us so it
    reaches full pstate before the first projection.
  - AV runs at lag 3 behind the energy groups and the last three AV groups
    of each block spill into the next block's first three slots, so the PE
    queue always has ready work and its idle gaps stay below the pstate
    reset threshold.
  - The residual x is re-read from the f32r SBUF projection operand via
    bitcast -- x is DMA'd once, not twice.
  - Tail (1/den, *gamma, +x, store) is split DVE/Pool so the two channel
    chunks normalize in parallel; within each AV pair the den matmul issues
    first so the final block's tail starts as early as possible.

PSUM budget (8 banks of 2KB): energy ring 2x[128,1024]f32 = 4, av accums
2x[128,512] = 2, den 1, projection scratch [128,512] = 1.
"""

import sys

if "/opt/trn_rl_repo" not in sys.path:
    sys.path.insert(0, "/opt/trn_rl_repo")

import numpy as np

import concourse.bacc as bacc
import concourse.mybir as mybir
import concourse.tile as tile
from concourse.bass_utils import run_bass_kernel_spmd

F32 = mybir.dt.float32
F32R = mybir.dt.float32r
BF16 = mybir.dt.bfloat16
FP8 = mybir.dt.float8e4

B, C, HW, D = 8, 256, 4096, 32
CH = C // 128
IBLK = 512
NIB = HW // IBLK
NJT = HW // 128
NPAIR = NJT // 2
NG = NJT // 4
NWARM = 10
EXPF = mybir.ActivationFunctionType.Exp
MULT = mybir.AluOpType.mult
ADD = mybir.AluOpType.add
DROW = mybir.MatmulPerfMode.DoubleRow


def _build():
    nc = bacc.Bacc("TRN2", target_bir_lowering=False, debug=False, num_devices=8)

    xr = nc.dram_tensor("xr", [C, HW], F32R, kind="ExternalInput")
    yr = nc.dram_tensor("yr", [C, HW], F32R, kind="ExternalInput")
    wq4d = nc.dram_tensor("wq4", [C, 128], F32R, kind="ExternalInput")
    wk4d = nc.dram_tensor("wk4", [C, 128], F32R, kind="ExternalInput")
    wvd = nc.dram_tensor("wvT", [C, C], F32R, kind="ExternalInput")
    bq4d = nc.dram_tensor("bq4", [128, 1], F32, kind="ExternalInput")
    bk4d = nc.dram_tensor("bk4", [128, 1], F32, kind="ExternalInput")
    gbvd = nc.dram_tensor("gbvd", [128, CH], F32, kind="ExternalInput")
    gmd = nc.dram_tensor("gmd", [128, 1], F32, kind="ExternalInput")
    g1md = nc.dram_tensor("g1md", [128, 1], F32, kind="ExternalInput")
    out = nc.dram_tensor("out", [C, HW], F32, kind="ExternalOutput")

    tc = tile.TileContext(nc)
    with tc:
        with (
            tc.tile_pool(name="cst", bufs=1) as cst,
            tc.tile_pool(name="io", bufs=1) as io,
            tc.tile_pool(name="qkv", bufs=1) as qkv,
        ):
            wq4_sb = cst.tile([128, CH * 128], F32R)
            wk4_sb = cst.tile([128, CH * 128], F32R)
            wv_sb = cst.tile([128, CH * C], F32R)
            bq4_sb = cst.tile([128, 1], F32)
            bk4_sb = cst.tile([128, 1], F32)
            gbv_sb = cst.tile([128, CH], F32)
            gm_sb = cst.tile([128, 1], F32)
            g1m_sb = cst.tile([128, 1], F32)
            ones_sb = cst.tile([128, 2 * 128], FP8)
            scr = cst.tile([1, 2], F32)

            xr_sb = io.tile([128, CH * HW], F32R)
            yr_sb = io.tile([128, CH * HW], F32R)
            q4 = qkv.tile([128, HW], BF16)
            k4 = qkv.tile([128, HW], BF16)
            vt = qkv.tile([128, NJT * C], FP8)

            # exp act-table preload: first act-engine instruction, no deps
            # beyond the memset, so LoadActFuncSet runs at t~0.  memset on
            # Pool (95ns) so the PE warmup below can start immediately.
            nc.gpsimd.memset(ones_sb[:], 1.0)
            nc.scalar.activation(scr[:, 0:1], ones_sb[0:1, 0:1], EXPF)

            # small constants via the gpsimd SWDGE queue; their transfers are
            # tiny and must not queue behind the big input transfers
            nc.gpsimd.dma_start(bk4_sb[:], bk4d[:])
            nc.gpsimd.dma_start(bq4_sb[:], bq4d[:])
            nc.gpsimd.dma_start(gbv_sb[:], gbvd[:])
            nc.gpsimd.dma_start(gm_sb[:], gmd[:])
            nc.gpsimd.dma_start(g1m_sb[:], g1md[:])

            # inputs on the sync/HWDGE queue in dependency-deadline order:
            # the first energy group needs wk4+y0 then wq4+x0; y chunk c
            # feeds k-chunk c (needed at block-0 slot c) and vt chunk c.
            yr3 = yr.rearrange("(ch p) N -> p ch N", ch=CH)
            xr3 = xr.rearrange("(ch p) N -> p ch N", ch=CH)
            yr_sb3 = yr_sb[:].rearrange("P (ch N) -> P ch N", ch=CH)
            xr_sb3 = xr_sb[:].rearrange("P (ch N) -> P ch N", ch=CH)

            def ld(dst3, src3, c0, c1):
                nc.sync.dma_start(dst3[:, :, c0:c1], src3[:, :, c0:c1])

            nc.sync.dma_start(wk4_sb[:, 0:128], wk4d[0:128, :])
            nc.sync.dma_start(wk4_sb[:, 128:256], wk4d[128:256, :])
            ld(yr_sb3, yr3, 0, IBLK)
            nc.sync.dma_start(wq4_sb[:, 0:128], wq4d[0:128, :])
            nc.sync.dma_start(wq4_sb[:, 128:256], wq4d[128:256, :])
            ld(xr_sb3, xr3, 0, IBLK)
            nc.sync.dma_start(wv_sb[:, 0:C], wvd[0:128, :])
            nc.sync.dma_start(wv_sb[:, C:2 * C], wvd[128:256, :])
            for c in range(1, NIB):
                ld(yr_sb3, yr3, c * IBLK, (c + 1) * IBLK)
            for n in range(1, NIB):
                ld(xr_sb3, xr3, n * IBLK, (n + 1) * IBLK)

            with (
                tc.tile_pool(name="ptp", bufs=2) as ptp,
                tc.tile_pool(name="wrk", bufs=2) as wrk,
                tc.tile_pool(name="psE", bufs=1, space="PSUM") as psE,
                tc.tile_pool(name="psAV", bufs=1, space="PSUM") as psAV,
                tc.tile_pool(name="psPR", bufs=1, space="PSUM") as psPR,
            ):
                psa = psPR.tile([128, IBLK], F32, name="psa")
                vrr = [0]
                kln = [0]
                loan = [True]

                def vt_region():
                    # scratch region for a vt tile.  While the av/den psum
                    # banks are still unused (before block 0's first av
                    # accumulation) they are loaned out, giving a depth-4
                    # rotation whose drains (Pool copies) never stall PE;
                    # afterwards the psa halves ping-pong (depth 2).
                    if loan[0]:
                        r = vrr[0] % 4
                        vrr[0] += 1
                        if r < 2:
                            return psa[:, r * 256:r * 256 + 256]
                        t = psAV.tile([128, IBLK], F32,
                                      name=f"vln_{vrr[0]}",
                                      tag="av0" if r == 2 else "av1")
                        return t[:, 0:256]
                    r = vrr[0] & 1
                    vrr[0] += 1
                    return psa[:, r * 256:r * 256 + 256]

                def kq_region():
                    if loan[0]:
                        kln[0] += 1
                        t = psAV.tile([128, IBLK], F32,
                                      name=f"kln_{kln[0]}", tag="den")
                        return t[:]
                    return psa[:]

                def k_chunk(c, w_sb=None, b_sb=None, src=None, dst=None):
                    w_sb = wk4_sb if w_sb is None else w_sb
                    b_sb = bk4_sb if b_sb is None else b_sb
                    src = yr_sb if src is None else src
                    dst = k4 if dst is None else dst
                    ps = kq_region()
                    for h in range(CH):
                        nc.tensor.matmul(
                            ps,
                            w_sb[:, h * 128:(h + 1) * 128],
                            src[:, h * HW + c * IBLK: h * HW + (c + 1) * IBLK],
                            start=(h == 0),
                            stop=(h == CH - 1),
                        )
                    nc.vector.tensor_scalar_add(
                        dst[:, c * IBLK:(c + 1) * IBLK], ps, b_sb[:, 0:1]
                    )

                def q_block(n):
                    k_chunk(n, wq4_sb, bq4_sb, xr_sb, q4)

                def vt_tile(jt):
                    ps = vt_region()
                    for h in range(CH):
                        nc.tensor.matmul(
                            ps,
                            yr_sb[:, h * HW + jt * 128: h * HW + (jt + 1) * 128],
                            wv_sb[:, h * C:(h + 1) * C],
                            start=(h == 0),
                            stop=(h == CH - 1),
                        )
                    # Pool cannot read PSUM on real hw; the fp8 cast-copy
                    # must run on DVE
                    nc.vector.tensor_copy(vt[:, jt * C:(jt + 1) * C], ps)

                # deadline-ordered projection filler, split into vt tiles
                # (interleaved between other PE work so the psum scratch
                # ping-pong latency hides) and k/q chunks.
                fill_vt = {}
                fill_kq = {}
                for g in range(4):
                    # loan window: 3 vt tiles per slot on the 4-deep scratch
                    fill_vt[(0, g)] = [3 * g, 3 * g + 1, 3 * g + 2]
                for g in range(4, 8):
                    fill_vt[(0, g)] = [2 * g + 4, 2 * g + 5]
                for g in range(4):
                    fill_vt[(1, g)] = [20 + 3 * g, 21 + 3 * g, 22 + 3 * g]
                for g in range(1, 8):
                    fill_kq[(0, g - 1)] = lambda g=g: k_chunk(g)
                fill_kq[(0, 7)] = lambda: q_block(1)
                fill_kq[(1, 4)] = lambda: q_block(2)
                for n in range(2, 7):
                    fill_kq[(n, 1)] = lambda n=n: q_block(n + 1)

                def et_group(n, g, pt):
                    # energy for (i-block n, group g): 4 row-packed K=32
                    # matmuls into two 2-bank psum tiles, then exp into pt
                    ets = [
                        psE.tile([128, 2 * IBLK], F32,
                                 name=f"et{h}_{n}_{g}", tag="et", bufs=2)
                        for h in range(2)
                    ]
                    for q in range(4):
                        jt = 4 * g + q
                        nc.tensor.matmul(
                            ets[q // 2][:, (q % 2) * IBLK:(q % 2 + 1) * IBLK],
                            k4[32 * q:32 * (q + 1), jt * 128:(jt + 1) * 128],
                            q4[32 * q:32 * (q + 1), n * IBLK:(n + 1) * IBLK],
                            start=True,
                            stop=True,
                            tile_position=(32 * q, 0),
                        )
                    for h in range(2):
                        nc.scalar.activation(
                            pt[:, (4 * g + 2 * h) * IBLK:(4 * g + 2 * h + 2) * IBLK],
                            ets[h][:], EXPF,
                        )

                ones_pair = ones_sb[:].rearrange("P (s c) -> P s c", s=2)

                def make_tail(n, get_avden, last=False):
                    def tail():
                        av, den = get_avden()
                        rgbg = wrk.tile([128, IBLK], F32, name=f"rgbg_{n}", tag="rgbg")
                        rgb = wrk.tile([128, IBLK], F32,
                                       name=f"rgb_{n}", tag="rgb")
                        nc.vector.reciprocal(rgb[:], den[:])
                        nc.vector.tensor_scalar(
                            rgbg[:], rgb[:], gm_sb[:, 0:1], None, MULT
                        )
                        ot = wrk.tile([128, CH * IBLK], F32, name=f"ot_{n}", tag="ot")
                        # av is PSUM so the tensor_tensor multiplies must run
                        # on DVE; the SBUF-only +bias+residual runs on Pool
                        for ch in range(CH):
                            xres = xr_sb[
                                :, ch * HW + n * IBLK: ch * HW + (n + 1) * IBLK
                            ].bitcast(F32)
                            tmp = wrk.tile([128, IBLK], F32,
                                           name=f"tmp_{n}_{ch}", tag=f"tmp{ch}")
                            nc.vector.tensor_tensor(tmp[:], av[ch][:], rgbg[:], MULT)
                            nc.gpsimd.scalar_tensor_tensor(
                                ot[:, ch * IBLK:(ch + 1) * IBLK],
                                tmp[:], gbv_sb[:, ch:ch + 1], xres, ADD, ADD,
                            )
                            # per-channel store so ch0 ships while ch1 runs
                            nc.sync.dma_start(
                                out[ch * 128:(ch + 1) * 128,
                                    n * IBLK:(n + 1) * IBLK],
                                ot[:, ch * IBLK:(ch + 1) * IBLK],
                            )
                    return tail

                prev_av = None
                prev_tail = None
                for n in range(NIB):
                    pt = ptp.tile([128, NJT * IBLK], FP8, name=f"pt_{n}", tag="pt")

                    # av/den accumulators are allocated lazily at the first
                    # av_pair so block 0's loan tiles (same tags) precede
                    # them in the ring's WAR chain
                    holder = {}

                    def get_avden(n=n, holder=holder):
                        if "av" not in holder:
                            holder["av"] = [
                                psAV.tile([128, IBLK], F32,
                                          name=f"av{ch}_{n}", tag=f"av{ch}")
                                for ch in range(CH)
                            ]
                            holder["den"] = psAV.tile(
                                [128, IBLK], F32, name=f"den_{n}", tag="den")
                        return holder["av"], holder["den"]

                    def av_pair(p, pt=pt, get=get_avden):
                        av, den = get()
                        # DoubleRow AV + denominator for j-tile pair p:
                        # virtual K=256 contracts two j-tiles at once.  den
                        # first so the last block's tail can start before
                        # its final av matmuls retire.
                        ptp_ap = pt[:, 2 * p * IBLK:(2 * p + 2) * IBLK].rearrange(
                            "P (s N) -> P s N", s=2
                        )
                        vtp_ap = vt[:, 2 * p * C:(2 * p + 2) * C].rearrange(
                            "P (s c) -> P s c", s=2
                        )
                        nc.tensor.matmul(
                            den[:],
                            ones_pair,
                            ptp_ap,
                            start=(p == 0),
                            stop=(p == NPAIR - 1),
                            perf_mode=DROW,
                            skip_group_check=True,
                        )
                        for ch in range(CH):
                            nc.tensor.matmul(
                                av[ch][:],
                                vtp_ap[:, :, ch * 128:(ch + 1) * 128],
                                ptp_ap,
                                start=(p == 0),
                                stop=(p == NPAIR - 1),
                                perf_mode=DROW,
                                skip_group_check=True,
                            )

                    def warm(k):
                        # PE pstate warmup burst into the energy psum ring:
                        # keeps the array streaming so the projections and
                        # first energy groups run at full clock
                        wt = psE.tile([128, 2 * IBLK], F32,
                                      name=f"warm_{k}", tag="et", bufs=2)
                        nc.tensor.matmul(
                            wt[:, 0:256], ones_sb[:, 0:128], ones_sb[:],
                            start=True, stop=True,
                        )

                    if n == 0:
                        for w in range(NWARM):
                            warm(w)
                        k_chunk(0)
                        for w in range(3):
                            warm(NWARM + w)
                        q_block(0)
                    for g in range(NG):
                        if n == 0 and g == 4:
                            loan[0] = False
                        et_group(n, g, pt)
                        kq = fill_kq.get((n, g))
                        seq = [kq] if kq else []
                        pairs = []
                        if n > 0 and g <= 3:
                            # spilled av groups 4..7 of the previous block
                            gg = NG - 4 + g
                            pairs = [(2 * gg, lambda f=prev_av, p=2 * gg: f(p)),
                                     (2 * gg + 1,
                                      lambda f=prev_av, p=2 * gg + 1: f(p))]
                        elif g >= 4:
                            gg = g - 4
                            pairs = [(2 * gg, lambda p=2 * gg: av_pair(p)),
                                     (2 * gg + 1,
                                      lambda p=2 * gg + 1: av_pair(p))]
                        vleft = [(j, (lambda j=j: vt_tile(j)))
                                 for j in fill_vt.get((n, g), ())]
                        # weave vt tiles between av pairs so each scratch
                        # half's drain is covered by non-psa matmul work;
                        # a pair's own j-tiles always emit before the pair
                        for p, pth in pairs:
                            tiles = (2 * p, 2 * p + 1)
                            seq += [th for j, th in vleft if j in tiles]
                            vleft = [(j, th) for j, th in vleft
                                     if j not in tiles]
                            seq.append(pth)
                            if vleft:
                                seq.append(vleft.pop(0)[1])
                        seq += [th for j, th in vleft]
                        for thunk in seq:
                            thunk()
                        if n > 0 and g == 3:
                            prev_tail()
                    prev_av = av_pair
                    prev_tail = make_tail(n, get_avden, last=(n == NIB - 1))
                for p in range(2 * (NG - 4), 2 * NG):
                    prev_av(p)
                prev_tail()
    nc.compile()
    return nc


_NC_CACHE = {}


def kernel(x, y, Wq, bq, Wk, bk, Wv, bv, gamma):
    assert x.shape == (B, C, 64, 64)
    xs = np.ascontiguousarray(x.reshape(B, C, HW).astype(np.float32))
    ys = np.ascontiguousarray(y.reshape(B, C, HW).astype(np.float32))
    wq4 = np.ascontiguousarray(np.tile(Wq.T.astype(np.float32), (1, 4)))
    wk4 = np.ascontiguousarray(np.tile(Wk.T.astype(np.float32), (1, 4)))
    wvT = np.ascontiguousarray(Wv.T.astype(np.float32))
    bq4 = np.ascontiguousarray(np.tile(bq.astype(np.float32), 4).reshape(128, 1))
    bk4 = np.ascontiguousarray(np.tile(bk.astype(np.float32), 4).reshape(128, 1))
    g = float(np.asarray(gamma).reshape(-1)[0])
    gbvh = np.ascontiguousarray((g * bv.astype(np.float32)).reshape(CH, 128).T)
    gmh = np.full((128, 1), g, dtype=np.float32)
    g1mh = np.full((128, 1), 1.0 / g if g != 0.0 else 0.0, dtype=np.float32)

    if "nc" not in _NC_CACHE:
        _NC_CACHE["nc"] = _build()
    nc = _NC_CACHE["nc"]

    in_maps = [
        {
            "xr": xs[b], "yr": ys[b],
            "wq4": wq4, "wk4": wk4, "wvT": wvT,
            "bq4": bq4, "bk4": bk4, "gbvd": gbvh, "gmd": gmh, "g1md": g1mh,
        }
        for b in range(B)
    ]
    res = run_bass_kernel_spmd(nc, in_maps, list(range(B)))
    outs = np.stack([res.results[b]["out"] for b in range(B)])
    return outs.reshape(B, C, 64, 64).astype(np.float32)
